# revision 47
# baseline (speedup 1.0000x reference)
"""Trainium2 Bass kernel for AnchorProcessor (nms_detection).

Input  x: [8, 255, 128, 128] f32.  Output: [8, 18, 128, 128] f32.
Strategy: shard along H across 8 cores (16 rows each). Each core's problem is
fully local (the buggy cross-batch max/argmax reduces over (N, cls) which are
both on-core), so there are no collectives.

Per core (N=8, A=3, cls=80, HL=16, W=128), score path works on NEGATED
scores (min-reduce) with a bit-packed value|index key:
  - logits preloaded per (anchor, 4-row block) as [80, N, 4, W] tiles with
    2KB DMA descriptors, alternating sync/scalar DMA queues.
  - PE transposes each (n, row) into PSUM [128pix x 80c].
  - negscore = lgps * (-obj): the per-partition-scalar multiply is split
    2/8 on ACT (per-n scalar.mul) and 6/8 on DVE (tensor_mul, broadcast).
  - pack (one J-batched DVE scalar_tensor_tensor):
      packed = (negscore & 0xFFFFFC00) | iota_bits   (iota = flat n*80+c)
    monotone in negscore (10-bit index in the cleared mantissa low bits),
    so ONE min-reduce yields both min(negscore) = -smax (quantized to
    2^-13 rel) and the argmax index in the low 10 bits.
  - tiny per-anchor extraction ops unpack smax / sarg; results are
    PE-transposed back and broadcast to all 8 batch entries.
(tensor_tensor_reduce with op1=max hangs on this silicon; op1=min fails
too - probed both. The eq+iota exact argmax costs a full extra DVE pass;
the packed quantization error (2^-13 rel on smax, ~1e-4 of pixels get a
tie-broken argmax) is far inside the 2e-2 gate.)
"""

import os
import sys

for _p in ("/opt/trn_rl_repo", "/root/.axon_site/_ro/trn_rl_repo"):
    if _p not in sys.path:
        sys.path.append(_p)

import numpy as np

from concourse import bacc, masks, mybir
from concourse.tile import TileContext

N = 8          # batch
A = 3          # anchors
CLS = 80       # classes per anchor
W = 128        # width
HL = 16        # local H rows per core (128 / 8 cores)
NCORES = 8

ANCHOR_W = (116.0, 156.0, 373.0)
ANCHOR_H = (90.0, 198.0, 326.0)

F32 = mybir.dt.float32
FR = mybir.dt.float32r
U32 = mybir.dt.uint32

N_ACT = 2      # batch entries whose obj-multiply runs on ACT (rest on DVE)


def build_nc(hl=HL, reps=1, use_ttr=False, n_act=N_ACT, fr=False):
    """Build the single-core graph (same SPMD graph on all 8 cores)."""
    import contextlib
    pix = hl * W           # pixels per core
    ch = hl                # one chunk per local h-row (128 pixels each)

    nc = bacc.Bacc("TRN2", target_bir_lowering=False, debug=False)

    x = nc.declare_dram_parameter("x", [N, 255, hl, W], F32, isOutput=False)
    grid = nc.declare_dram_parameter("grid", [2, A * N, pix], F32, isOutput=False)
    anch = nc.declare_dram_parameter("anch", [2, A * N, 1], F32, isOutput=False)
    iota = nc.declare_dram_parameter("iota", [N * CLS], U32, isOutput=False)
    bits = nc.declare_dram_parameter("bits", [4], U32, isOutput=False)
    out = nc.declare_dram_parameter("out", [N, A * 6, hl, W], F32, isOutput=True)
    oscr = nc.dram_tensor("oscratch", [A * 2, hl, W], F32)

    with TileContext(nc) as tc:
        with (
            tc.tile_pool(name="const", bufs=1) as constp,
            tc.tile_pool(name="box", bufs=2) as boxp,
            tc.tile_pool(name="objsb", bufs=1) as objsbp,
            tc.tile_pool(name="lg", bufs=4) as lgp,
            tc.tile_pool(name="score", bufs=2) as scorep,
            tc.tile_pool(name="res", bufs=2) as resp,
            tc.tile_pool(name="outsb", bufs=3) as outsbp,
            tc.tile_pool(name="ps", bufs=3, space="PSUM") as psp,
            tc.tile_pool(name="ps2", bufs=1, space="PSUM") as ps2p,
        ):
            ident = constp.tile([128, 128], F32)
            masks.make_identity(nc, ident[:, :])

            gridt = [constp.tile([A * N, pix], F32, name=f"grid{g}", tag=f"grid{g}") for g in range(2)]
            ancht = [constp.tile([A * N, 1], F32, name=f"anch{g}", tag=f"anch{g}") for g in range(2)]
            for g in range(2):
                nc.scalar.dma_start(out=gridt[g][:, :], in_=grid[g, :, :])
                nc.scalar.dma_start(out=ancht[g][:, :], in_=anch[g, :, :])

            iotat = constp.tile([128, N * CLS], U32)
            nc.scalar.dma_start(
                out=iotat[:, :],
                in_=iota[:].unsqueeze(0).broadcast_to([128, N * CLS]),
            )
            bitst = constp.tile([128, 4], U32)
            nc.scalar.dma_start(
                out=bitst[:, :],
                in_=bits[:].unsqueeze(0).broadcast_to([128, 4]),
            )

            loop_cm = (
                tc.For_i(0, reps, 1, hint_engines=(mybir.EngineType.PE,))
                if reps > 1 else contextlib.nullcontext()
            )
            with loop_cm:
                body(nc, tc, x, out, oscr, pix, ch, hl,
                     ident, gridt, ancht, iotat, bitst, n_act, fr,
                     constp, boxp, objsbp, lgp, scorep, resp, outsbp, psp, ps2p)

    nc.compile()
    return nc


def body(nc, tc, x, out, oscr, pix, ch, hl, ident, gridt, ancht, iotat, bitst,
         n_act, fr,
         constp, boxp, objsbp, lgp, scorep, resp, outsbp, psp, ps2p):
    # objectness planes, rows ordered (a, n) a-major (scalar queue: keep the
    # sync queue free so the first logit block lands ASAP)
    objt = boxp.tile([A * N, pix], F32, tag="objt", name="objt")
    nc.scalar.dma_start(
        out=objt[:, :],
        in_=x[:, 4:255:85, :, :].transpose([1, 0, 2, 3]),
    )

    # NEGATED transposed objectness: objTn[pix, chunk, a, n] = -obj
    objTn = objsbp.tile([128, ch, A, N], F32)
    for j in range(ch):
        ops = ps2p.tile([128, A * N], F32)
        nc.tensor.transpose(
            ops[:, :], objt[:, j * 128:(j + 1) * 128], ident[:A * N, :A * N]
        )
        nc.scalar.mul(objTn[:, j, :, :], ops[:, :], -1.0)

    # ---------------- score path (negated, bit-packed argmax) --------------
    HB = 4 if ch % 4 == 0 else ch      # rows per logit preload block
    JB = 4 if ch % 4 == 0 else 1       # rows per pack/reduce batch
    for a in range(A):
        negsmaxT = resp.tile([128, ch], F32, tag="smaxT")
        for hb in range(0, ch, HB):
            # preload logits for all n, HB rows: 2KB contiguous descriptors
            lg = lgp.tile([80, N, HB, W], F32)
            eng = nc.sync if (hb // HB) % 2 == 0 else nc.scalar
            eng.dma_start(
                out=lg[:, :, :, :],
                in_=x[:, a * 85 + 5:a * 85 + 85, hb:hb + HB, :].transpose(
                    [1, 0, 2, 3]),
            )
            for rg in range(0, HB, JB):
                scoreg = scorep.tile([128, JB, N, CLS], F32, tag="negscore")
                packed = scorep.tile([128, JB, N, CLS], F32, tag="packed")
                for jj in range(JB):
                    r = rg + jj
                    j = hb + r
                    # transpose each n into PSUM: lgps[pix, n, c]
                    # (fp32r transpose mode: 1.5 PE cycles/row vs 2.0 fp32)
                    lgps = psp.tile([128, N, 128], F32)
                    for n in (0, 4, 1, 5, 2, 6, 3, 7):
                        if fr:
                            nc.tensor.transpose(
                                lgps[:, n, 0:80].bitcast(FR),
                                lg[:, n, r, :].bitcast(FR),
                                ident[:80, :80].bitcast(FR),
                            )
                        else:
                            nc.tensor.transpose(
                                lgps[:, n, 0:80], lg[:, n, r, :],
                                ident[:80, :80]
                            )
                    # negscore = lgps * (-obj): ACT for n < n_act (per-n
                    # per-partition scalar), DVE for the rest (broadcast).
                    for n in range(n_act):
                        nc.scalar.mul(
                            scoreg[:, jj, n, :], lgps[:, n, 0:80],
                            objTn[:, j, a, n:n + 1],
                        )
                    if n_act < N:
                        nobj_b = objTn[:, j, a, n_act:].unsqueeze(2).broadcast_to(
                            [128, N - n_act, CLS])
                        nc.vector.tensor_mul(
                            scoreg[:, jj, n_act:, :],
                            lgps[:, n_act:, 0:80], nobj_b)
                # pack = (negscore | 0x3FF) ^ (0x3FF ^ iota_bits)  (J-batched)
                # == (negscore & ~0x3FF) | iota, without NaN constants
                nc.vector.scalar_tensor_tensor(
                    out=packed[:, :, :, :].bitcast(U32),
                    in0=scoreg[:, :, :, :].bitcast(U32),
                    scalar=bitst[:, 0:1],
                    in1=iotat[:, :].rearrange(
                        "p (n c) -> p n c", n=N).unsqueeze(1).broadcast_to(
                        [128, JB, N, CLS]),
                    op0=mybir.AluOpType.bitwise_or,
                    op1=mybir.AluOpType.bitwise_xor,
                )
                nc.vector.tensor_reduce(
                    negsmaxT[:, hb + rg:hb + rg + JB],
                    packed[:, :, :, :].rearrange("p j n c -> p j (n c)"),
                    axis=mybir.AxisListType.X,
                    op=mybir.AluOpType.min,
                )

        # unpack: vq = (packed | 0x3FF) ^ 0x3FF  (= packed & ~0x3FF);
        # sarg = (((packed & 0x3FF) | bits(1.0)) - 1.0) * 2^23
        vq = resp.tile([128, ch], F32, tag="vq")
        nc.vector.scalar_tensor_tensor(
            out=vq[:, :].bitcast(U32), in0=negsmaxT[:, :].bitcast(U32),
            scalar=bitst[:, 0:1],
            in1=bitst[:, 0:1].broadcast_to([128, ch]),
            op0=mybir.AluOpType.bitwise_or, op1=mybir.AluOpType.bitwise_xor,
        )
        sargT = resp.tile([128, ch], F32, tag="sargT")
        nc.vector.scalar_tensor_tensor(
            out=sargT[:, :].bitcast(U32), in0=negsmaxT[:, :].bitcast(U32),
            scalar=bitst[:, 0:1],
            in1=bitst[:, 1:2].broadcast_to([128, ch]),
            op0=mybir.AluOpType.bitwise_and, op1=mybir.AluOpType.bitwise_or,
        )
        nc.vector.scalar_tensor_tensor(
            out=sargT[:, :], in0=sargT[:, :], scalar=1.0,
            in1=bitst[:, 2:3].bitcast(F32).broadcast_to([128, ch]),
            op0=mybir.AluOpType.subtract, op1=mybir.AluOpType.mult,
        )

        for t_in, ch_out, scl in ((vq, a * 6 + 4, -1.0),
                                  (sargT, a * 6 + 5, 1.0)):
            tps = ps2p.tile([hl, 128], F32, tag="outps")
            nc.tensor.transpose(tps[:, :], t_in[:, :], ident[:, :])
            osb = outsbp.tile([hl, 128], F32, tag="osb")
            if scl == 1.0:
                nc.scalar.copy(osb[:, :], tps[:, :])
            else:
                nc.scalar.mul(osb[:, :], tps[:, :], scl)
            si = (ch_out % 6 - 4) * A + a
            nc.sync.dma_start(out=oscr[si, :, :], in_=osb[:, :])
            nc.sync.dma_start(
                out=out[:, ch_out, :, :],
                in_=oscr[si, :, :].unsqueeze(0).broadcast_to(
                    [N, hl, W]),
            )

    # ---------------- box path (natural layout) ----------------
    for k in (0, 1, 2, 3):
        t = boxp.tile([A * N, pix], F32, tag="boxt", name=f"bx{k}")
        nc.scalar.dma_start(
            out=t[:, :],
            in_=x[:, k:255:85, :, :].transpose([1, 0, 2, 3]),
        )
        o = boxp.tile([A * N, pix], F32, tag="boxo", name=f"bo{k}")
        if k < 2:
            nc.scalar.activation(
                o[:, :], t[:, :], mybir.ActivationFunctionType.Sigmoid
            )
            # + gx (rows 0..23) or + gy (rows 24..47) on the idle gpsimd
            nc.vector.tensor_add(o[:, :], o[:, :], gridt[k][:, :])
        else:
            # per-partition anchor const via ACT scale
            nc.scalar.mul(o[:, :], t[:, :], ancht[k - 2][:, :])
        nc.sync.dma_start(
            out=out[:, k:18:6, :, :].transpose([1, 0, 2, 3]),
            in_=o[:, :],
        )


_NC_CACHE = {}


def get_nc(hl=HL, n_act=None, fr=None):
    if n_act is None:
        n_act = int(os.environ.get("NACT", str(N_ACT)))
    if fr is None:
        fr = os.environ.get("FR", "0") == "1"
    key = (hl, n_act, fr)
    if key not in _NC_CACHE:
        _NC_CACHE[key] = build_nc(hl, n_act=n_act, fr=fr)
    return _NC_CACHE[key]


def make_in_maps(x, hl=HL):
    """Shard the full input along H and build per-core input maps."""
    x = np.ascontiguousarray(x, dtype=np.float32)
    pix = hl * W
    gx = np.tile(np.arange(W, dtype=np.float32), hl)          # value = w
    anch_col = np.stack(
        [np.repeat(np.array(ANCHOR_W, np.float32), N),
         np.repeat(np.array(ANCHOR_H, np.float32), N)]
    ).reshape(2, A * N, 1)
    iota_bits = np.arange(N * CLS, dtype=np.uint32) ^ 0x3FF
    bits = np.array([0x3FF, 0x3F800000, 0x4B000000, 0],
                    np.uint32)  # masklo, bits(1.0), bits(2^23), unused
    in_maps = []
    ncores = x.shape[2] // hl
    for i in range(ncores):
        gy = np.repeat(np.arange(i * hl, (i + 1) * hl, dtype=np.float32), W)
        grid = np.empty((2, A * N, pix), np.float32)
        grid[0] = gx
        grid[1] = gy
        in_maps.append({
            "x": np.ascontiguousarray(x[:, :, i * hl:(i + 1) * hl, :]),
            "grid": grid,
            "anch": anch_col,
            "iota": iota_bits,
            "bits": bits,
        })
    return in_maps


def patch_compile_cache(cache_dir="/tmp/bass_neff_cache"):
    """Cache compiled NEFFs on disk keyed by the BIR hash (compile takes
    minutes; the cache makes repeated runs of an identical graph instant)."""
    import hashlib
    import shutil
    import concourse.bass2jax as b2j

    if getattr(b2j, "_neff_cache_patched", False):
        return
    os.makedirs(cache_dir, exist_ok=True)
    orig = b2j.compile_bir_kernel

    def cached(bir_json, tmpdir, neff_name="file.neff"):
        data = bir_json if isinstance(bir_json, bytes) else str(bir_json).encode()
        key = hashlib.sha256(data).hexdigest()[:32]
        cpath = os.path.join(cache_dir, key + ".neff")
        if os.path.exists(cpath):
            opath = os.path.join(tmpdir, neff_name)
            shutil.copy(cpath, opath)
            return opath
        r = orig(bir_json, tmpdir, neff_name)
        try:
            shutil.copy(r, cpath)
        except OSError:
            pass
        return r

    b2j.compile_bir_kernel = cached
    b2j._neff_cache_patched = True


def kernel(x: np.ndarray) -> np.ndarray:
    from concourse.bass_utils import run_bass_kernel_spmd

    patch_compile_cache()

    nc = get_nc(HL)
    in_maps = make_in_maps(x, HL)
    res = run_bass_kernel_spmd(nc, in_maps, core_ids=list(range(NCORES)))
    return np.concatenate([res.results[i]["out"] for i in range(NCORES)], axis=2)

